# revision 1
# baseline (speedup 1.0000x reference)
"""Emformer attention Bass/Tile kernel for 8 Trainium2 NeuronCores.

Strategy: data-parallel over batch (B=16 -> 2 batches per core). Everything on
device is computed in a transposed layout so that no on-device transposes are
needed anywhere:

  qT  [D, Q] = Wq  @ q_in^T        (per head h: qT[h]  = [d=128, Q])
  kT  [D, K] = Wk  @ kv_in^T       (per head h: kT[h]  = [d=128, K])
  v   [K, D] = kv_in^T^T @ Wv^T    (per K-tile: [128, D])
  sT  [K, Q] = kT[h]^T-slices @ qT[h]   (PE lhsT = kT slice, rhs = qT)
  pT  [K, Q] = exp(SCALE*sT + padbias) * mask01T     (softmax numerator)
  den [1, Q] = ones^T @ pT                            (softmax denominator)
  aT  [d, Q] = v-slice^T @ pT  accumulated over K-tiles, then * 1/den
  oT  [D, Q] = Wo^T-slices^T @ aT

The attention mask and the per-sample key-padding mask are folded into a
multiplicative 0/1 mask (exp of a *real* score times zero == contribution of a
-1e8 masked score after softmax, exactly) plus a -10000 additive bias on padded
key partitions, so no NEG_INF arithmetic is needed on device.
"""

import os
import sys
import types
from contextlib import ExitStack

import numpy as np
import ml_dtypes

import concourse.bass as bass
import concourse.bacc as bacc
import concourse.mybir as mybir
import concourse.tile as tile
from concourse.bass_utils import run_bass_kernel_spmd

BF16 = ml_dtypes.bfloat16

# Problem constants (hardcoded per spec)
D = 1024
H = 8
d = D // H  # 128
T = 1024
R = 32
S = 8
M = 8
B = 16
Q = R + T + S   # 1064
K = M + R + T   # 1064
NCORES = 8
BPC = B // NCORES  # batches per core = 2
SCALE = float(d) ** -0.5
NKT = (K + 127) // 128          # 9 K-tiles (last has 40 rows)
QCH = [(0, 512), (512, 512), (1024, Q - 1024)]   # query chunks (PSUM bank = 512 fp32)
NM = D // 128                   # 8 row-blocks of the D dimension

_BF = mybir.dt.bfloat16
_F32 = mybir.dt.float32


def _build_program(has_bq, has_bk, has_bv, has_bo):
    nc = bacc.Bacc("TRN2", target_bir_lowering=False, debug=False,
                   enable_asserts=True, num_devices=NCORES)

    qinT_d = nc.dram_tensor("qinT", [BPC, NM, 128, Q], _BF, kind="ExternalInput").ap()
    kvinT_d = nc.dram_tensor("kvinT", [BPC, NM, 128, K], _BF, kind="ExternalInput").ap()
    wq_d = nc.dram_tensor("wq", [NM, 128, D], _BF, kind="ExternalInput").ap()
    wk_d = nc.dram_tensor("wk", [NM, 128, D], _BF, kind="ExternalInput").ap()
    wv_d = nc.dram_tensor("wv", [NM, 128, D], _BF, kind="ExternalInput").ap()
    wo_d = nc.dram_tensor("wo", [NM, 128, D], _BF, kind="ExternalInput").ap()
    m01_d = nc.dram_tensor("m01", [NKT, 128, Q], _BF, kind="ExternalInput").ap()
    padb_d = nc.dram_tensor("padb", [128, BPC * NKT], _F32, kind="ExternalInput").ap()
    if has_bq:
        bq_d = nc.dram_tensor("bq", [128, NM], _F32, kind="ExternalInput").ap()
    if has_bk:
        bk_d = nc.dram_tensor("bk", [128, NM], _F32, kind="ExternalInput").ap()
    if has_bv:
        bvb_d = nc.dram_tensor("bvb", [128, D], _F32, kind="ExternalInput").ap()
    if has_bo:
        bo_d = nc.dram_tensor("bo", [128, NM], _F32, kind="ExternalInput").ap()
    outT_d = nc.dram_tensor("outT", [BPC, D, Q], _F32, kind="ExternalOutput").ap()

    AF = mybir.ActivationFunctionType

    with tile.TileContext(nc) as tc, ExitStack() as ctx:
        # ---- persistent tiles -------------------------------------------
        sbp = ctx.enter_context(tc.tile_pool(name="persist", bufs=1))
        m01t = [sbp.tile([128, Q], _BF, name=f"m01t{i}") for i in range(NKT)]
        for i in range(NKT):
            nc.sync.dma_start(m01t[i][:], m01_d[i])
        padbt = sbp.tile([128, BPC * NKT], _F32, name="padbt")
        nc.sync.dma_start(padbt[:], padb_d)
        ones = sbp.tile([128, 1], _BF, name="ones")
        nc.vector.memset(ones[:], 1.0)
        if has_bq:
            bqt = sbp.tile([128, NM], _F32, name="bqt")
            nc.sync.dma_start(bqt[:], bq_d)
        if has_bk:
            bkt = sbp.tile([128, NM], _F32, name="bkt")
            nc.sync.dma_start(bkt[:], bk_d)
        if has_bv:
            bvbt = sbp.tile([128, D], _F32, name="bvbt")
            nc.sync.dma_start(bvbt[:], bvb_d)
        if has_bo:
            bot = sbp.tile([128, NM], _F32, name="bot")
            nc.sync.dma_start(bot[:], bo_d)

        qTt = [sbp.tile([128, Q], _BF, name=f"qTt{i}") for i in range(NM)]
        kTt = [sbp.tile([128, Q], _BF, name=f"kTt{i}") for i in range(NM)]
        vt = [sbp.tile([128, D], _BF, name=f"vt{i}") for i in range(NKT)]
        attn = [sbp.tile([128, Q], _BF, name=f"attn{i}") for i in range(NM)]
        den_b = sbp.tile([H, Q], _F32, name="den_b")
        rec = sbp.tile([H, Q], _F32, name="rec")
        probs8 = sbp.tile([128, 512], _BF, name="probs8")
        # last K-tile has only K-1024 live rows; zero once so stale rows
        # never reach PE as garbage (they multiply zero probs / zero mask)
        nc.vector.memset(probs8[:], 0.0)
        nc.vector.memset(vt[NKT - 1][:], 0.0)

        # ---- pools -------------------------------------------------------
        inp = ctx.enter_context(tc.tile_pool(name="inp", bufs=NM))
        wpool = ctx.enter_context(tc.tile_pool(name="wpool", bufs=2 * NM))
        ppool = ctx.enter_context(tc.tile_pool(name="ppool", bufs=12))
        ostage = ctx.enter_context(tc.tile_pool(name="ostage", bufs=3))
        dstage = ctx.enter_context(tc.tile_pool(name="dstage", bufs=2))
        rpool = ctx.enter_context(tc.tile_pool(name="rpool", bufs=1))
        bcpool = ctx.enter_context(tc.tile_pool(name="bcpool", bufs=1))
        ps_sp = ctx.enter_context(tc.tile_pool(name="ps_sp", bufs=3, space="PSUM"))
        ps_o = ctx.enter_context(tc.tile_pool(name="ps_o", bufs=2, space="PSUM"))
        ps_d = ctx.enter_context(tc.tile_pool(name="ps_d", bufs=2, space="PSUM"))

        def load_w(dram):
            tiles = []
            for i in range(NM):
                wt = wpool.tile([128, D], _BF, tag="w")
                nc.sync.dma_start(wt[:], dram[i])
                tiles.append(wt)
            return tiles

        def proj_T(in_tiles, w_tiles, out_tiles, bias_tile):
            # out[m] [128, Q] = sum_kc w[kc][:, m-block].T @ in[kc][:, qchunk]
            for m in range(NM):
                for (qo, qw) in QCH:
                    ps = ps_sp.tile([128, qw], _F32, tag="sp")
                    for kc in range(NM):
                        nc.tensor.matmul(
                            ps[:], w_tiles[kc][:, m * 128:(m + 1) * 128],
                            in_tiles[kc][:, qo:qo + qw],
                            start=(kc == 0), stop=(kc == NM - 1))
                    if bias_tile is not None:
                        nc.scalar.activation(out_tiles[m][:, qo:qo + qw], ps[:],
                                             AF.Identity, bias=bias_tile[:, m:m + 1])
                    else:
                        nc.scalar.copy(out_tiles[m][:, qo:qo + qw], ps[:])

        for b in range(BPC):
            # ---- phase A: load inputs + projections ----------------------
            qin = []
            for i in range(NM):
                t = inp.tile([128, Q], _BF, tag="in")
                nc.sync.dma_start(t[:], qinT_d[b, i])
                qin.append(t)
            wq_t = load_w(wq_d)
            wk_t = load_w(wk_d)
            proj_T(qin, wq_t, qTt, bqt if has_bq else None)
            kvin = []
            for i in range(NM):
                t = inp.tile([128, K], _BF, tag="in")
                nc.sync.dma_start(t[:], kvinT_d[b, i])
                kvin.append(t)
            proj_T(kvin, wk_t, kTt, bkt if has_bk else None)
            wv_t = load_w(wv_d)
            # v[kt] [pw, D] = sum_kc kvin[kc][:, ktile].T @ wv[kc][:, nchunk]
            for kt in range(NKT):
                pw = min(128, K - kt * 128)
                for no in range(0, D, 512):
                    ps = ps_sp.tile([128, 512], _F32, tag="sp")
                    for kc in range(NM):
                        nc.tensor.matmul(
                            ps[0:pw, :], kvin[kc][:, kt * 128:kt * 128 + pw],
                            wv_t[kc][:, no:no + 512],
                            start=(kc == 0), stop=(kc == NM - 1))
                    nc.scalar.copy(vt[kt][0:pw, no:no + 512], ps[0:pw, :])
                    if has_bv:
                        nc.vector.tensor_add(vt[kt][0:pw, no:no + 512],
                                             vt[kt][0:pw, no:no + 512],
                                             bvbt[0:pw, no:no + 512])

            # ---- phase B: attention per head -----------------------------
            for h in range(H):
                for (qo, qw) in QCH:
                    probs = []
                    for kt in range(NKT):
                        pw = min(128, K - kt * 128)
                        ps = ps_sp.tile([128, qw], _F32, tag="sp")
                        nc.tensor.matmul(
                            ps[0:pw, 0:qw], kTt[h][:, kt * 128:kt * 128 + pw],
                            qTt[h][:, qo:qo + qw], start=True, stop=True)
                        if kt == NKT - 1:
                            pt = probs8
                        else:
                            pt = ppool.tile([128, 512], _BF, tag="p")
                        nc.scalar.activation(
                            pt[0:pw, 0:qw], ps[0:pw, 0:qw], AF.Exp,
                            bias=padbt[0:pw, b * NKT + kt:b * NKT + kt + 1],
                            scale=SCALE)
                        nc.vector.tensor_mul(pt[0:pw, 0:qw], pt[0:pw, 0:qw],
                                             m01t[kt][0:pw, qo:qo + qw])
                        probs.append(pt)
                    o_ps = ps_o.tile([128, qw], _F32, tag="o")
                    d_ps = ps_d.tile([1, qw], _F32, tag="d")
                    for kt in range(NKT):
                        nc.tensor.matmul(
                            o_ps[:, 0:qw], vt[kt][:, h * 128:(h + 1) * 128],
                            probs[kt][:, 0:qw],
                            start=(kt == 0), stop=(kt == NKT - 1))
                        nc.tensor.matmul(
                            d_ps[:, 0:qw], ones[:],
                            probs[kt][:, 0:qw],
                            start=(kt == 0), stop=(kt == NKT - 1))
                    nc.scalar.copy(attn[h][:, qo:qo + qw], o_ps[:, 0:qw])
                    dst = dstage.tile([1, qw], _F32, tag="ds")
                    nc.scalar.copy(dst[:], d_ps[:, 0:qw])
                    nc.sync.dma_start(den_b[h:h + 1, qo:qo + qw], dst[:])

            # ---- phase C: normalize --------------------------------------
            nc.vector.reciprocal(rec[:], den_b[:])
            for h in range(H):
                rt = rpool.tile([1, Q], _F32, tag="rt")
                nc.sync.dma_start(rt[:], rec[h:h + 1, :])
                bc = bcpool.tile([128, Q], _F32, tag="bc")
                nc.gpsimd.partition_broadcast(bc[:], rt[:])
                nc.vector.tensor_mul(attn[h][:], attn[h][:], bc[:])

            # ---- phase D: output projection ------------------------------
            wo_t = load_w(wo_d)
            for m in range(NM):
                for (qo, qw) in QCH:
                    ps = ps_sp.tile([128, qw], _F32, tag="sp")
                    for kc in range(NM):
                        nc.tensor.matmul(
                            ps[:, 0:qw], wo_t[kc][:, m * 128:(m + 1) * 128],
                            attn[kc][:, qo:qo + qw],
                            start=(kc == 0), stop=(kc == NM - 1))
                    ot = ostage.tile([128, qw], _F32, tag="os")
                    if has_bo:
                        nc.scalar.activation(ot[:, 0:qw], ps[:, 0:qw], AF.Identity,
                                             bias=bot[:, m:m + 1])
                    else:
                        nc.vector.tensor_copy(ot[:, 0:qw], ps[:, 0:qw])
                    nc.sync.dma_start(outT_d[b, m * 128:(m + 1) * 128, qo:qo + qw],
                                      ot[:, 0:qw])

    nc.compile()
    return nc


_prog_cache = {}


def _get_program(key):
    if key not in _prog_cache:
        _prog_cache[key] = _build_program(*key)
    return _prog_cache[key]


def kernel(utterance, lengths, right_context, summary, mems, attention_mask,
           Wq, bq, Wkv, bkv, Wo, bo):
    utterance = np.asarray(utterance, np.float32)
    right_context = np.asarray(right_context, np.float32)
    summary = np.asarray(summary, np.float32)
    mems = np.asarray(mems, np.float32)
    lengths = np.asarray(lengths)
    attention_mask = np.asarray(attention_mask)
    Wq = np.asarray(Wq, np.float32)
    Wkv = np.asarray(Wkv, np.float32)
    Wo = np.asarray(Wo, np.float32)
    bq = np.asarray(bq, np.float32)
    bkv = np.asarray(bkv, np.float32)
    bo = np.asarray(bo, np.float32)

    # ---- host-side prep (layouts, masks) ---------------------------------
    q_in = np.concatenate([right_context, utterance, summary], axis=0)   # (Q,B,D)
    kv_in = np.concatenate([mems, right_context, utterance], axis=0)     # (K,B,D)
    # transposed per-batch layouts, tiled on 128-partition row blocks
    qinT = np.ascontiguousarray(q_in.transpose(2, 1, 0)).astype(BF16)    # (D,B,Q)
    kvinT = np.ascontiguousarray(kv_in.transpose(2, 1, 0)).astype(BF16)  # (D,B,K)

    wq_h = np.ascontiguousarray(Wq.T).reshape(NM, 128, D).astype(BF16)
    wk_h = np.ascontiguousarray(Wkv[:D].T).reshape(NM, 128, D).astype(BF16)
    wv_h = np.ascontiguousarray(Wkv[D:].T).reshape(NM, 128, D).astype(BF16)
    wo_h = np.ascontiguousarray(Wo.T).reshape(NM, 128, D).astype(BF16)

    m01 = (~attention_mask).T.astype(BF16)                                # (K,Q)
    m01_p = np.zeros((NKT * 128, Q), BF16)
    m01_p[:K] = m01
    m01_h = m01_p.reshape(NKT, 128, Q)

    rcbl = Q - int(lengths.max()) - S
    klengths = (lengths.astype(np.int64) + M + rcbl).astype(np.int64)    # (B,)

    has_bq = bool(np.any(bq))
    has_bk = bool(np.any(bkv[:D]))
    has_bv = bool(np.any(bkv[D:]))
    has_bo = bool(np.any(bo))

    nc = _get_program((has_bq, has_bk, has_bv, has_bo))

    gidx = np.arange(NKT * 128)
    in_maps = []
    for c in range(NCORES):
        bs = [c * BPC + j for j in range(BPC)]
        padb = np.zeros((128, BPC * NKT), np.float32)
        for j, bb in enumerate(bs):
            pb = np.where(gidx >= klengths[bb], np.float32(-10000.0), np.float32(0.0))
            padb[:, j * NKT:(j + 1) * NKT] = pb.reshape(NKT, 128).T
        im = {
            "qinT": np.ascontiguousarray(
                qinT[:, bs, :].transpose(1, 0, 2).reshape(BPC, NM, 128, Q)),
            "kvinT": np.ascontiguousarray(
                kvinT[:, bs, :].transpose(1, 0, 2).reshape(BPC, NM, 128, K)),
            "wq": wq_h, "wk": wk_h, "wv": wv_h, "wo": wo_h,
            "m01": m01_h, "padb": padb,
        }
        if has_bq:
            im["bq"] = bq.reshape(NM, 128).T.copy()
        if has_bk:
            im["bk"] = bkv[:D].reshape(NM, 128).T.copy()
        if has_bv:
            im["bvb"] = np.broadcast_to(bkv[D:], (128, D)).copy()
        if has_bo:
            im["bo"] = bo.reshape(NM, 128).T.copy()
        in_maps.append(im)

    res = run_bass_kernel_spmd(nc, in_maps, list(range(NCORES)))

    # ---- gather + unshard -------------------------------------------------
    out = np.empty((Q, B, D), np.float32)
    for c in range(NCORES):
        oT = res.results[c]["outT"]                      # (BPC, D, Q)
        for j in range(BPC):
            out[:, c * BPC + j, :] = oT[j].T
    output = out[:Q - S]                                 # (R+T, B, D)
    out_mems = np.clip(out[Q - S:], -10.0, 10.0)[:-1]    # (S-1, B, D)
    return output, out_mems
